# revision 18
# baseline (speedup 1.0000x reference)
"""Trainium2 Bass kernel for one NCA (neural cellular automata) step.

Reference computation (B=32, H=W=256, C=16, HID=128):
  alive  = x[..., 3] > 0.1
  nsum   = 3x3 box-sum of alive (zero pad)
  nb     = 3x3x16 neighborhood gather (zero pad)      [B,H,W,144]
  h      = relu(nb @ W1 + b1)                          [B,H,W,128]
  upd    = h @ W2 + b2                                 [B,H,W,16]
  upd    = where(nsum == 0, 0, upd)
  out    = clip(x + upd, -1, 1)

Strategy: batch-parallel over 8 cores (4 images each). On each core the
image data is kept channel-major ("xT" layout, [16, npix]) over a
zero-padded 258x258 per-image grid, so every 3x3 shift is a constant
free-dim offset and zero padding is exact with no edge fixups.

Per 512-pixel tile: layer 1 = 3 accumulating K=48 bf16 matmuls (the 3
row-shifts are pre-stacked on partitions by the load DMA; the 3 column
shifts are free-dim offsets 0/1/2). Tiles are processed in pairs using
PE row-tiling (partition bases 0 and 64) and in superblocks of 4 tiles
for the layer-2 output packing (M=16 at PSUM partition bases 0/32/64/96).
The alive/nsum mask is computed densely (128 tiles per instruction) on
the vector engine and replicated to the packed layout with a selector
matmul. Epilogue (mask mul, +x in fp32, clip) runs on DVE; relu+b1 on ACT.
"""
import os
import sys

if "/opt/trn_rl_repo" not in sys.path:
    sys.path.insert(0, "/opt/trn_rl_repo")

import numpy as np
import ml_dtypes

import concourse.bacc as bacc
import concourse.bass as bass
import concourse.mybir as mybir
import concourse.tile as tile
from concourse.bass_utils import run_bass_kernel_spmd

BF16 = ml_dtypes.bfloat16

# ---- problem constants ----
B, H, W, C = 32, 256, 256, 16
HID = 128
NCORES = 8
BC = B // NCORES          # images per core
PH, PW = H + 2, W + 2     # padded image dims
IMG = PH * PW             # padded pixels per image
NPIX = BC * IMG           # padded pixels per core (266256)
TILE = 512                # pixels per tile (one PSUM bank of fp32)
SB = 4                    # tiles per superblock
GUARD = 512               # zero guard columns before the data
NT = -(-NPIX // TILE)     # 521 tiles of real data
NSB = -(-NT // SB)        # superblocks
NT_FULL = NSB * SB        # tiles incl. tail garbage
COLS = GUARD + NT_FULL * TILE + 1024  # xT columns incl. guard/slack
MBLOCKS = -(-NT_FULL // 128)         # mask blocks (128 tiles each)

ALIVE_THRESHOLD = 0.1

_CACHE = {}


def _build(n_sb, cols, b1_zero, b2_zero):
    f32 = mybir.dt.float32
    bf16 = mybir.dt.bfloat16
    relu = mybir.ActivationFunctionType.Relu
    ident = mybir.ActivationFunctionType.Identity
    gt = mybir.AluOpType.is_gt
    n_mb = -(-n_sb * SB // 128)

    nc = bacc.Bacc("TRN2", target_bir_lowering=False, debug=False,
                   num_devices=NCORES)
    xf = nc.dram_tensor("xf", [C, cols], f32, kind="ExternalInput")
    xb3 = nc.dram_tensor("xb3", [48, cols], bf16, kind="ExternalInput")
    w1q = nc.dram_tensor("w1q", [128, 3, 128], bf16, kind="ExternalInput")
    w2 = nc.dram_tensor("w2", [128, 16], bf16, kind="ExternalInput")
    b1v = nc.dram_tensor("b1v", [128, 1], f32, kind="ExternalInput")
    b2v = nc.dram_tensor("b2v", [128, 1], f32, kind="ExternalInput")
    selw = nc.dram_tensor("selw", [128, 32, 128], bf16, kind="ExternalInput")
    y = nc.dram_tensor("y", [C, cols], f32, kind="ExternalOutput")

    with tile.TileContext(nc) as tc:
        with (
            tc.tile_pool(name="singles", bufs=1) as singles,
            tc.tile_pool(name="loads", bufs=3) as loads,
            tc.tile_pool(name="mask", bufs=2) as maskp,
            tc.tile_pool(name="work", bufs=3) as work,
            tc.tile_pool(name="psum_h", bufs=3, space="PSUM") as psum_h,
            tc.tile_pool(name="psum_u", bufs=1, space="PSUM") as psum_u,
        ):
            # ---- constants into SBUF ----
            w1q_sb = singles.tile([128, 3, 128], bf16)
            nc.sync.dma_start(out=w1q_sb[:], in_=w1q[:])
            w2_sb = singles.tile([128, 16], bf16)
            nc.sync.dma_start(out=w2_sb[:], in_=w2[:])
            selw_sb = singles.tile([128, 32, 128], bf16)
            nc.sync.dma_start(out=selw_sb[:], in_=selw[:])
            b1_sb = b2_sb = None
            if not b1_zero:
                b1_sb = singles.tile([128, 1], f32)
                nc.sync.dma_start(out=b1_sb[:], in_=b1v[:])
            if not b2_zero:
                b2_sb = singles.tile([128, 1], f32)
                nc.sync.dma_start(out=b2_sb[:], in_=b2v[:])

            mb_cur = None
            for s in range(n_sb):
                p_in_block = s % 32
                if p_in_block == 0:
                    # ---- dense mask pipeline for the next 128 tiles ----
                    blk = s // 32
                    t0 = blk * 128
                    nrows = min(128, n_sb * SB - t0)
                    base = GUARD + t0 * TILE - 259
                    msrc = maskp.tile([128, 1032], f32, tag="msrc")
                    src = bass.AP(tensor=xf, offset=3 * cols + base,
                                  ap=[[TILE, nrows], [1, 1032]])
                    nc.sync.dma_start(out=msrc[:nrows, :], in_=src)
                    alive = maskp.tile([128, 1032], f32, tag="alive")
                    nc.vector.tensor_scalar(alive[:nrows], msrc[:nrows],
                                            ALIVE_THRESHOLD, None, op0=gt)
                    rsum = maskp.tile([128, 516], f32, tag="rsum")
                    nc.vector.tensor_add(rsum[:nrows], alive[:nrows, 0:516],
                                         alive[:nrows, 258:774])
                    nc.vector.tensor_add(rsum[:nrows], rsum[:nrows],
                                         alive[:nrows, 516:1032])
                    nsum = maskp.tile([128, 514], f32, tag="nsum")
                    nc.vector.tensor_add(nsum[:nrows], rsum[:nrows, 0:514],
                                         rsum[:nrows, 1:515])
                    nc.vector.tensor_add(nsum[:nrows, 0:512], nsum[:nrows, 0:512],
                                         rsum[:nrows, 2:514])
                    mb_cur = maskp.tile([128, 512], bf16, tag="mb")
                    if nrows < 128:
                        nc.vector.memset(mb_cur[:], 0.0)
                    nc.vector.tensor_scalar(mb_cur[:nrows], nsum[:nrows, 0:512],
                                            0.5, None, op0=gt)

                n0 = GUARD + s * SB * TILE
                # ---- neighborhood row-shift stack: rows 0-47 window @n0,
                # rows 64-111 the same window shifted +512 (for row-tiling) ---
                QW = 3 * TILE + 4
                qa = loads.tile([128, QW], bf16, tag="qa")
                for g in range(2):
                    nc.sync.dma_start(
                        out=qa[64 * g:64 * g + 48, :],
                        in_=bass.AP(tensor=xb3, offset=n0 - 1 + g * TILE,
                                    ap=[[cols, 48], [1, QW]]))
                # ---- fp32 x in the packed epilogue layout ----
                x4 = loads.tile([128, TILE], f32, tag="x4")
                for g in range(4):
                    nc.sync.dma_start(
                        out=x4[32 * g:32 * g + 16, :],
                        in_=bass.AP(tensor=xf, offset=n0 + g * TILE,
                                    ap=[[cols, 16], [1, TILE]]))

                # ---- layer 1: 3 accumulating K=48 matmuls per tile; tiles
                # run in pairs on PE row-groups 0-47 / 64-111 ----
                hts = []
                for pair in range(2):
                    ht = psum_h.tile([128, 2 * TILE], f32, tag="ht")
                    for dj in range(3):
                        for g in range(2):
                            rb = 64 * g
                            nc.tensor.matmul(
                                out=ht[:, g * TILE:(g + 1) * TILE],
                                lhsT=w1q_sb[rb:rb + 48, dj, :],
                                rhs=qa[rb:rb + 48, pair * 2 * TILE + dj:
                                       pair * 2 * TILE + dj + TILE],
                                start=(dj == 0), stop=(dj == 2),
                                tile_position=(rb, 0))
                    # ---- relu + b1, cast to bf16 ----
                    hs = work.tile([128, 2 * TILE], bf16, tag="hs")
                    if b1_zero:
                        nc.scalar.activation(hs[:], ht[:], relu)
                    else:
                        nc.scalar.activation(hs[:], ht[:], relu, bias=b1_sb[:])
                    hts.append(hs)

                # ---- mask replication matmul ----
                mask4 = psum_u.tile([128, TILE], f32, tag="mask4")
                nc.tensor.matmul(out=mask4[:], lhsT=selw_sb[:, p_in_block, :],
                                 rhs=mb_cur[:], start=True, stop=True)
                # ---- layer 2: 4 col-packed M=16 matmuls ----
                upd4 = psum_u.tile([128, TILE], f32, tag="upd4")
                for g in range(4):
                    nc.tensor.matmul(
                        out=upd4[32 * g:32 * g + 16, :], lhsT=w2_sb[:],
                        rhs=hts[g // 2][:, (g % 2) * TILE:(g % 2 + 1) * TILE],
                        start=True, stop=True, tile_position=(0, 32 * g))

                # ---- epilogue: out = clip(x + mask*(upd+b2), -1, 1) ----
                msb = work.tile([128, TILE], f32, tag="msb")
                nc.scalar.activation(msb[:], mask4[:], ident)
                if not b2_zero:
                    ub = work.tile([128, TILE], f32, tag="ub")
                    nc.scalar.activation(ub[:], upd4[:], ident, bias=b2_sb[:])
                    usrc = ub
                else:
                    usrc = upd4
                u = work.tile([128, TILE], f32, tag="u")
                nc.vector.tensor_mul(u[:], usrc[:], msb[:])
                v = work.tile([128, TILE], f32, tag="v")
                nc.vector.tensor_add(v[:], u[:], x4[:])
                o4 = work.tile([128, TILE], f32, tag="o4")
                nc.vector.tensor_scalar(o4[:], v[:], 1.0, -1.0,
                                        op0=mybir.AluOpType.min,
                                        op1=mybir.AluOpType.max)
                for g in range(4):
                    nc.sync.dma_start(
                        out=bass.AP(tensor=y, offset=n0 + g * TILE,
                                    ap=[[cols, 16], [1, TILE]]),
                        in_=o4[32 * g:32 * g + 16, :])
    nc.compile()
    return nc


def _prep_core(x_shard):
    """x_shard [BC,H,W,C] fp32 -> (xf [C,COLS] fp32, xb3 [48,COLS] bf16).

    xb3 row 16*di+c holds channel c shifted by (di-1)*PW (the 3 row shifts
    of the 3x3 neighborhood, pre-stacked for the K=48 matmuls)."""
    xp = np.zeros((BC, PH, PW, C), np.float32)
    xp[:, 1:PH - 1, 1:PW - 1, :] = x_shard
    xt = xp.transpose(3, 0, 1, 2).reshape(C, NPIX)
    xf = np.zeros((C, COLS), np.float32)
    xf[:, GUARD:GUARD + NPIX] = xt
    xfb = xf.astype(BF16)
    xb3 = np.zeros((48, COLS), BF16)
    for di in range(3):
        sh = (di - 1) * PW
        lo, hi = max(0, sh), COLS + min(0, sh)
        xb3[16 * di:16 * di + 16, lo - sh:hi - sh] = xfb[:, lo:hi]
    return xf, xb3


def _prep_weights(W1, b1, W2, b2):
    w1q = np.zeros((128, 3, 128), np.float32)
    for dj in range(3):
        for di in range(3):
            blk = W1[(3 * di + dj) * 16:(3 * di + dj) * 16 + 16, :]
            w1q[16 * di:16 * di + 16, dj, :] = blk
            w1q[64 + 16 * di:64 + 16 * di + 16, dj, :] = blk
    selw = np.zeros((128, 32, 128), np.float32)
    for p in range(32):
        for g in range(4):
            for r in range(16):
                selw[4 * p + g, p, 32 * g + r] = 1.0
    b2v = np.zeros((128, 1), np.float32)
    for g in range(4):
        b2v[32 * g:32 * g + 16, 0] = b2
    return (w1q.astype(BF16), W2.astype(BF16),
            b1.reshape(128, 1).astype(np.float32), b2v, selw.astype(BF16))


def kernel(x, W1, b1, W2, b2):
    x = np.asarray(x, np.float32)
    W1 = np.asarray(W1, np.float32)
    b1 = np.asarray(b1, np.float32)
    W2 = np.asarray(W2, np.float32)
    b2 = np.asarray(b2, np.float32)

    b1_zero = not np.any(b1)
    b2_zero = not np.any(b2)
    key = ("nca", NSB, COLS, b1_zero, b2_zero)
    if key not in _CACHE:
        _CACHE[key] = _build(NSB, COLS, b1_zero, b2_zero)
    nc = _CACHE[key]

    w1qb, w2b, b1v, b2v, selwb = _prep_weights(W1, b1, W2, b2)
    in_maps = []
    for i in range(NCORES):
        xf, xb3 = _prep_core(x[i * BC:(i + 1) * BC])
        in_maps.append({"xf": xf, "xb3": xb3, "w1q": w1qb, "w2": w2b,
                       "b1v": b1v, "b2v": b2v, "selw": selwb})
    res = run_bass_kernel_spmd(nc, in_maps, core_ids=list(range(NCORES)))

    out = np.empty((B, H, W, C), np.float32)
    for i in range(NCORES):
        yt = res.results[i]["y"][:, GUARD:GUARD + NPIX]
        yp = yt.reshape(C, BC, PH, PW).transpose(1, 2, 3, 0)
        out[i * BC:(i + 1) * BC] = yp[:, 1:PH - 1, 1:PW - 1, :]
    return out
